# revision 33
# baseline (speedup 1.0000x reference)
import numpy as np

B = 8
SEQ = 4096
D = 1024
N_BASE = 10000.0
N_CORES = 8
SPC = SEQ // N_CORES  # seq rows per core (512)
U = SPC // 128        # seq rows per partition (4)

_CACHE = {}


def _compute_pe() -> np.ndarray:
    """Mirror of the reference _pos_encoding (default jax backend, f32)."""
    import jax
    import jax.numpy as jnp

    pos = jnp.arange(SEQ, dtype=jnp.float32)[:, None]
    i = jnp.arange(D // 2, dtype=jnp.float32)
    denom = jnp.power(jnp.float32(N_BASE), 2.0 * i / jnp.float32(D))
    ang = pos / denom
    pe = jnp.stack([jnp.sin(ang), jnp.cos(ang)], axis=-1).reshape(SEQ, D)
    return np.asarray(jax.device_get(pe), dtype=np.float32)


def _build_program():
    import concourse.bacc as bacc
    import concourse.mybir as mybir
    import concourse.tile as tile

    nc = bacc.Bacc("TRN2")
    f32 = mybir.dt.float32
    x_in = nc.declare_dram_parameter("x", [B * SPC, D], f32, isOutput=False)
    pe_in = nc.declare_dram_parameter("pe", [SPC, D], f32, isOutput=False)
    y_out = nc.declare_dram_parameter("y", [B * SPC, D], f32, isOutput=True)

    hu = U // 2
    with tile.TileContext(nc) as tc:
        with (
            tc.tile_pool(name="pe_pool", bufs=1) as pe_pool,
            tc.tile_pool(name="x_pool", bufs=B - 2) as x_pool,
            tc.tile_pool(name="sub_pool", bufs=4) as sub_pool,
        ):
            pe_t = pe_pool.tile([128, U, D], f32)
            pe_ap = pe_in.rearrange("(p u) d -> p u d", u=U)
            # pe rides the (initially idle) scalar/output queue so the
            # x stream starts on the sync queue at t=0.
            nc.scalar.dma_start(out=pe_t[:], in_=pe_ap)
            # x0, x2-x6: one whole-tile DMA each on the sync ring; x1 is
            # split into two half sub-tiles, one per ring, which balances
            # both rings at exactly 17.85MB and gives the scalar ring
            # input work during the ramp-in.
            xts = []
            x1h = []
            for b in range(B - 1):
                xs = x_in[b * SPC : (b + 1) * SPC, :].rearrange(
                    "(p u) d -> p u d", u=U
                )
                if b == 1:
                    for hi, eng in enumerate((nc.sync, nc.scalar)):
                        xt = sub_pool.tile([128, hu, D], f32)
                        eng.dma_start(
                            out=xt[:], in_=xs[:, hi * hu : (hi + 1) * hu, :]
                        )
                        x1h.append(xt)
                    xts.append(None)
                    continue
                xt = x_pool.tile([128, U, D], f32)
                nc.sync.dma_start(out=xt[:], in_=xs)
                xts.append(xt)
            # x7: two half sub-tiles (whole-tile DMAs; the column slice
            # lives only on the dependency-free DRAM access pattern), so
            # the final add is 2.2us instead of 4.4
            x7ap = x_in[(B - 1) * SPC : B * SPC, :].rearrange(
                "(p u) d -> p u d", u=U
            )
            x7h = []
            for hi in range(2):
                xt = sub_pool.tile([128, hu, D], f32)
                nc.sync.dma_start(
                    out=xt[:], in_=x7ap[:, hi * hu : (hi + 1) * hu, :]
                )
                x7h.append(xt)
            # whole-tile adds; y0-y6 on the scalar ring behind pe
            for b in range(B - 1):
                ys = y_out[b * SPC : (b + 1) * SPC, :].rearrange(
                    "(p u) d -> p u d", u=U
                )
                if b == 1:
                    for hi in range(2):
                        nc.vector.tensor_add(
                            x1h[hi][:], x1h[hi][:],
                            pe_t[:, hi * hu : (hi + 1) * hu, :],
                        )
                        nc.scalar.dma_start(
                            out=ys[:, hi * hu : (hi + 1) * hu, :],
                            in_=x1h[hi][:],
                        )
                    continue
                nc.vector.tensor_add(xts[b][:], xts[b][:], pe_t[:])
                nc.scalar.dma_start(out=ys, in_=xts[b][:])
            # y7 halves ride the SYNC ring (idle once x is in): both
            # rings drain the output backlog together at the end.
            y7ap = y_out[(B - 1) * SPC : B * SPC, :].rearrange(
                "(p u) d -> p u d", u=U
            )
            for hi in range(2):
                nc.vector.tensor_add(
                    x7h[hi][:], x7h[hi][:],
                    pe_t[:, hi * hu : (hi + 1) * hu, :],
                )
                nc.sync.dma_start(
                    out=y7ap[:, hi * hu : (hi + 1) * hu, :], in_=x7h[hi][:]
                )
    if not nc.is_finalized():
        nc.finalize()
    return nc


def _get_state():
    if "nc" not in _CACHE:
        _CACHE["nc"] = _build_program()
    if "pe" not in _CACHE:
        _CACHE["pe"] = _compute_pe()
    return _CACHE["nc"], _CACHE["pe"]


def _in_maps(x, pe):
    in_maps = []
    for c in range(N_CORES):
        xs = np.ascontiguousarray(x[:, c * SPC : (c + 1) * SPC, :]).reshape(
            B * SPC, D
        )
        pes = np.ascontiguousarray(pe[c * SPC : (c + 1) * SPC, :])
        in_maps.append({"x": xs, "pe": pes})
    return in_maps


def kernel(x, seq_len=None, **_):
    from concourse.bass_utils import run_bass_kernel_spmd

    x = np.asarray(x, dtype=np.float32)
    assert x.shape == (B, SEQ, D)
    if seq_len is not None:
        assert int(np.asarray(seq_len)) == SEQ

    nc, pe = _get_state()
    res = run_bass_kernel_spmd(nc, _in_maps(x, pe), list(range(N_CORES))).results

    out = np.empty((B, SEQ, D), dtype=np.float32)
    for c in range(N_CORES):
        out[:, c * SPC : (c + 1) * SPC, :] = res[c]["y"].reshape(B, SPC, D)
    return out


# revision 34
# speedup vs baseline: 1.1791x; 1.1791x over previous
import numpy as np

B = 8
SEQ = 4096
D = 1024
N_BASE = 10000.0
N_CORES = 8
SPC = SEQ // N_CORES  # seq rows per core (512)
U = SPC // 128        # seq rows per partition (4)

_CACHE = {}


def _compute_pe() -> np.ndarray:
    """Mirror of the reference _pos_encoding (default jax backend, f32)."""
    import jax
    import jax.numpy as jnp

    pos = jnp.arange(SEQ, dtype=jnp.float32)[:, None]
    i = jnp.arange(D // 2, dtype=jnp.float32)
    denom = jnp.power(jnp.float32(N_BASE), 2.0 * i / jnp.float32(D))
    ang = pos / denom
    pe = jnp.stack([jnp.sin(ang), jnp.cos(ang)], axis=-1).reshape(SEQ, D)
    return np.asarray(jax.device_get(pe), dtype=np.float32)


def _build_program():
    import concourse.bacc as bacc
    import concourse.mybir as mybir
    import concourse.tile as tile

    nc = bacc.Bacc("TRN2")
    f32 = mybir.dt.float32
    x_in = nc.declare_dram_parameter("x", [B * SPC, D], f32, isOutput=False)
    pe_in = nc.declare_dram_parameter("pe", [SPC, D], f32, isOutput=False)
    y_out = nc.declare_dram_parameter("y", [B * SPC, D], f32, isOutput=True)

    hu = U // 2
    with tile.TileContext(nc) as tc:
        with (
            tc.tile_pool(name="pe_pool", bufs=1) as pe_pool,
            tc.tile_pool(name="x_pool", bufs=B - 1) as x_pool,
            tc.tile_pool(name="sub_pool", bufs=2) as sub_pool,
        ):
            pe_t = pe_pool.tile([128, U, D], f32)
            pe_ap = pe_in.rearrange("(p u) d -> p u d", u=U)
            # pe rides the (initially idle) scalar/output queue so the
            # x stream starts on the sync queue at t=0.
            nc.scalar.dma_start(out=pe_t[:], in_=pe_ap)
            # x0-x6: one whole-tile DMA each on the sync ring
            xts = []
            for b in range(B - 1):
                xs = x_in[b * SPC : (b + 1) * SPC, :].rearrange(
                    "(p u) d -> p u d", u=U
                )
                xt = x_pool.tile([128, U, D], f32)
                nc.sync.dma_start(out=xt[:], in_=xs)
                xts.append(xt)
            # x7: two half sub-tiles (whole-tile DMAs; the column slice
            # lives only on the dependency-free DRAM access pattern), so
            # the final add is 2.2us instead of 4.4
            x7ap = x_in[(B - 1) * SPC : B * SPC, :].rearrange(
                "(p u) d -> p u d", u=U
            )
            x7h = []
            for hi in range(2):
                xt = sub_pool.tile([128, hu, D], f32)
                nc.sync.dma_start(
                    out=xt[:], in_=x7ap[:, hi * hu : (hi + 1) * hu, :]
                )
                x7h.append(xt)
            # whole-tile adds; y0-y6 on the scalar ring behind pe
            for b in range(B - 1):
                nc.vector.tensor_add(xts[b][:], xts[b][:], pe_t[:])
                ys = y_out[b * SPC : (b + 1) * SPC, :].rearrange(
                    "(p u) d -> p u d", u=U
                )
                nc.scalar.dma_start(out=ys, in_=xts[b][:])
            # y7 halves ride the SYNC ring (idle once x is in): both
            # rings drain the output backlog together at the end.
            y7ap = y_out[(B - 1) * SPC : B * SPC, :].rearrange(
                "(p u) d -> p u d", u=U
            )
            for hi in range(2):
                nc.vector.tensor_add(
                    x7h[hi][:], x7h[hi][:],
                    pe_t[:, hi * hu : (hi + 1) * hu, :],
                )
                nc.sync.dma_start(
                    out=y7ap[:, hi * hu : (hi + 1) * hu, :], in_=x7h[hi][:]
                )
    if not nc.is_finalized():
        nc.finalize()
    return nc


def _get_state():
    if "nc" not in _CACHE:
        _CACHE["nc"] = _build_program()
    if "pe" not in _CACHE:
        _CACHE["pe"] = _compute_pe()
    return _CACHE["nc"], _CACHE["pe"]


def _in_maps(x, pe):
    in_maps = []
    for c in range(N_CORES):
        xs = np.ascontiguousarray(x[:, c * SPC : (c + 1) * SPC, :]).reshape(
            B * SPC, D
        )
        pes = np.ascontiguousarray(pe[c * SPC : (c + 1) * SPC, :])
        in_maps.append({"x": xs, "pe": pes})
    return in_maps


def kernel(x, seq_len=None, **_):
    from concourse.bass_utils import run_bass_kernel_spmd

    x = np.asarray(x, dtype=np.float32)
    assert x.shape == (B, SEQ, D)
    if seq_len is not None:
        assert int(np.asarray(seq_len)) == SEQ

    nc, pe = _get_state()
    res = run_bass_kernel_spmd(nc, _in_maps(x, pe), list(range(N_CORES))).results

    out = np.empty((B, SEQ, D), dtype=np.float32)
    for c in range(N_CORES):
        out[:, c * SPC : (c + 1) * SPC, :] = res[c]["y"].reshape(B, SPC, D)
    return out
